# revision 1
# baseline (speedup 1.0000x reference)
"""Trainium2 Bass kernel for nn_Critic (2-layer GATv2 + TopK pooling critic).

Sharding: data-parallel over the B=32 graph dimension - 4 graphs per core on
8 NeuronCores. The dense per-node projections (x @ Wl, x @ Wr for both GAT
layers - the bulk of the dense FLOPs) run on device via a Bass/Tile program
executed with run_bass_kernel_spmd; edge gather/scatter, softmax, topk
selection and the tiny readout MLP run on host, exactly mirroring the
reference semantics (including top-k tie-breaking).

Self-contained: hardcodes all shapes; no repo-local imports.
"""
import concurrent.futures as _fut

import numpy as np
import scipy.sparse as _sp

import concourse.bacc as bacc
import concourse.mybir as mybir
import concourse.tile as tile
import concourse.bass_utils as bass_utils

B, N, DEG = 32, 1024, 8
E = B * N * DEG
NT = B * N
F_IN, HID, EDIM, NR, HD = 64, 128, 16, 16, 4
NEG = 0.2
K1 = 820
K2 = 656
CORES = 8
GPC = B // CORES
NLOC = GPC * N          # 4096 nodes per core
HC = HD * HID           # 512

_F32 = mybir.dt.float32
_PROG = None


def _build_program():
    """One Bass program, run SPMD on 8 cores: for this core's 4096 nodes,
    compute xl = x @ Wl (contraction dim padded to 128)."""
    nc = bacc.Bacc("TRN2", target_bir_lowering=False, debug=False)
    xT = nc.dram_tensor("xT", [128, NLOC], _F32, kind="ExternalInput")
    wl = nc.dram_tensor("wl", [128, HC], _F32, kind="ExternalInput")
    xl = nc.dram_tensor("xl", [NLOC, HC], _F32, kind="ExternalOutput")

    with tile.TileContext(nc) as tc:
        with tc.tile_pool(name="wp", bufs=1) as wp, \
             tc.tile_pool(name="sb", bufs=4) as sb, \
             tc.tile_pool(name="ps", bufs=4, space="PSUM") as ps:
            wl_sb = wp.tile([128, HC], _F32)
            nc.sync.dma_start(out=wl_sb[:], in_=wl[:])
            xT_sb = wp.tile([128, NLOC], _F32)
            nc.sync.dma_start(out=xT_sb[:], in_=xT[:])
            for t in range(NLOC // 128):
                cols = slice(t * 128, (t + 1) * 128)
                pt = ps.tile([128, HC], _F32)
                nc.tensor.matmul(pt[:], lhsT=xT_sb[:, cols], rhs=wl_sb[:],
                                 start=True, stop=True)
                ot = sb.tile([128, HC], _F32)
                nc.vector.tensor_copy(out=ot[:], in_=pt[:])
                nc.sync.dma_start(out=xl[cols, :], in_=ot[:])
    nc.compile()
    return nc


_EXEC = _fut.ThreadPoolExecutor(max_workers=1)


def _device_xl_submit(feats, Wl):
    """Launch the 8-core SPMD xl = feats @ Wl dispatch asynchronously.
    The axon RPC wait releases the GIL, so host numpy overlaps with it."""
    global _PROG
    if _PROG is None:
        _PROG = _build_program()
    F = feats.shape[1]
    wl_p = np.zeros((128, HC), np.float32)
    wl_p[:F] = Wl
    in_maps = []
    for c in range(CORES):
        xT = np.zeros((128, NLOC), np.float32)
        xT[:F] = feats[c * NLOC:(c + 1) * NLOC].T
        in_maps.append({"xT": np.ascontiguousarray(xT), "wl": wl_p})

    def run():
        res = bass_utils.run_bass_kernel_spmd(
            _PROG, in_maps, core_ids=list(range(CORES)), trace=False)
        return np.concatenate([res.results[c]["xl"] for c in range(CORES)],
                              axis=0)
    return _EXEC.submit(run)


_SCRATCH = None


def _gatv2_host(xl_fut, bl, xr, ea_s, We, seg, em_s, att, bias):
    """Host mirror of the reference GATv2 on dst-sorted edges.

    xl_fut: future for the device xl projection (pre-bias) - the
    xl-independent half (xr gather, edge-feature GEMM) runs while the device
    dispatch is in flight. seg = (src_s, dst_s, starts, empty, ST, tperm);
    ea_s / em_s already in sorted order; ST is the [NT, E] csr scatter
    matrix. Uses lrelu(x) = 0.6x + 0.4|x| so the head-wise attention dot
    becomes two BLAS GEMMs, scipy spmm for the output segment-sum, and
    preallocated scratch (fresh 537MB numpy allocations page-fault at
    ~600MB/s on this 1-cpu host, which previously dominated the runtime)."""
    global _SCRATCH
    if _SCRATCH is None:
        _SCRATCH = tuple(np.empty((E, HC), np.float32) for _ in range(3))
    xl_src, m, scr = _SCRATCH
    src_s, dst_s, starts, empty, ST, tperm = seg
    np.take(xr, dst_s, axis=0, out=m)                   # overlaps device xl
    np.matmul(ea_s, We, out=scr)                        # ef, sorted order
    m += scr
    xl = xl_fut.result()                                # join device dispatch
    xl += bl[None, :].astype(np.float32)
    np.take(xl, src_s, axis=0, out=xl_src)              # kept for scatter pass
    m += xl_src
    attW = np.zeros((HC, HD), np.float32)               # block-diag att
    for h in range(HD):
        attW[h * HID:(h + 1) * HID, h] = att[h]
    am = np.abs(m, out=scr)
    logits = np.float32(0.5 * (1 + NEG)) * (m @ attW)
    logits += np.float32(0.5 * (1 - NEG)) * (am @ attW)
    logits = np.where(em_s[:, None], logits, np.float32(-1e9))
    # softmax shift cancels in alpha = a/den; logits are O(1) so exp is safe
    # (masked edges: exp(-1e9) underflows to exactly 0).
    a = np.exp(logits)
    den = np.add.reduceat(a, starts, axis=0)
    den[empty] = 0
    alpha = a / (den[dst_s] + np.float32(1e-16))
    # scatter: out[:, h] block = (ST * alpha_h) @ xl_src_h via csr data swap
    out = np.empty((NT, HC), np.float32)
    for h in range(HD):
        ST.data = alpha[tperm, h]
        out[:, h * HID:(h + 1) * HID] = ST @ xl_src[:, h * HID:(h + 1) * HID]
    h_nodes = out.reshape(NT, HD, HID).mean(axis=1) + bias[None, :].astype(np.float32)
    return np.maximum(h_nodes, np.float32(0))


def _topk_host(h, node_mask, p, k):
    score = (h @ p.astype(np.float32)) / np.float32(np.linalg.norm(p) + 1e-16)
    gate = np.tanh(score)
    s = np.where(node_mask, score, -np.inf).reshape(B, N)
    # jax.lax.top_k semantics: k largest, ties broken toward lower index
    idx = np.argsort(-s, axis=1, kind="stable")[:, :k]
    keep = np.zeros((B, N), bool)
    np.put_along_axis(keep, idx, True, axis=1)
    return h * gate[:, None], keep.reshape(-1)


def kernel(x, edge_attr, action, W1l, b1l, W1r, b1r, W1e, att1, bias1,
           W2l, b2l, W2r, b2r, W2e, att2, bias2, p1, p2,
           Wf1, bf1, Wf2, bf2, Wf3, bf3, edge_index):
    f32 = np.float32
    x = np.asarray(x, f32)
    edge_attr = np.asarray(edge_attr, f32)
    action = np.asarray(action, f32)
    edge_index = np.asarray(edge_index)
    src, dst = edge_index[0].astype(np.int64), edge_index[1].astype(np.int64)
    args = {k: np.asarray(v, f32) for k, v in dict(
        W1l=W1l, b1l=b1l, W1r=W1r, b1r=b1r, W1e=W1e, att1=att1, bias1=bias1,
        W2l=W2l, b2l=b2l, W2r=W2r, b2r=b2r, W2e=W2e, att2=att2, bias2=bias2,
        p1=p1, p2=p2, Wf1=Wf1, bf1=bf1, Wf2=Wf2, bf2=bf2, Wf3=Wf3,
        bf3=bf3).items()}

    # ---- layer 1: launch device xl dispatch, overlap host-side prep ----
    fut1 = _device_xl_submit(x, args["W1l"])

    # dst-sorted edge structures (shared by both layers) - overlaps dispatch
    order = np.argsort(dst, kind="stable")
    src_s, dst_s = src[order], dst[order]
    starts = np.minimum(np.searchsorted(dst_s, np.arange(NT)), E - 1)
    empty = np.bincount(dst_s, minlength=NT) == 0
    S = _sp.csr_matrix((np.arange(E, dtype=np.float64), dst_s,
                        np.arange(E + 1)), shape=(E, NT))
    ST = S.T.tocsr()
    tperm = ST.data.astype(np.int64)
    ST.data = np.ones(E, np.float32)
    seg = (src_s, dst_s, starts, empty, ST, tperm)
    ea_s = edge_attr[order]

    xr1 = x @ args["W1r"] + args["b1r"]
    em0_s = np.ones(E, bool)
    h1 = _gatv2_host(fut1, args["b1l"], xr1, ea_s, args["W1e"], seg, em0_s,
                     args["att1"], args["bias1"])
    h1, keep1 = _topk_host(h1, np.ones(NT, bool), args["p1"], K1)

    # ---- layer 2 ----
    fut2 = _device_xl_submit(h1, args["W2l"])
    em1_s = keep1[src_s] & keep1[dst_s]
    xr2 = h1 @ args["W2r"] + args["b2r"]
    h2 = _gatv2_host(fut2, args["b2l"], xr2, ea_s, args["W2e"], seg, em1_s,
                     args["att2"], args["bias2"])
    h2, keep2 = _topk_host(h2, keep1, args["p2"], K2)

    # ---- readout ----
    hb = h2.reshape(B, N, HID)
    mb = keep2.reshape(B, N)[..., None]
    gmx = np.where(mb, hb, -np.inf).max(axis=1)
    gav = (hb * mb).sum(axis=1) / np.float32(K2)
    z = np.concatenate([gmx, gav, action], axis=1)
    z = np.maximum(z @ args["Wf1"] + args["bf1"], 0)
    z = np.maximum(z @ args["Wf2"] + args["bf2"], 0)
    return (z @ args["Wf3"] + args["bf3"]).astype(np.float32)



# revision 3
# speedup vs baseline: 11.1149x; 11.1149x over previous
"""Trainium2 Bass kernel for nn_Critic (2-layer GATv2 + TopK pooling critic).

Sharding: data-parallel over the B=32 graph dimension - 4 graphs per core on 8
NeuronCores. The ENTIRE GNN forward (both GATv2 layers, on-device TopK pooling,
global max/mean pooling) runs in ONE SPMD dispatch; the host only quantizes
inputs to int16 (halving the axon-tunnel upload, which dominates wall time),
builds wrapped index arrays for the on-device gather/scatter DMAs, and runs the
tiny [32,272] readout MLP on the pooled outputs.

Device-side GATv2 (per core, 4096 nodes / 32768 edges):
  A. xl/xr node projections + ef edge projections on TensorE; xl/xr written to
     HBM tables with a mask-bias column (576-float rows for the 256B DMA rule).
  B. Edge loop: gpsimd.dma_gather of endpoint rows, lrelu(xl_s+xr_d+ef) via
     0.6x+0.4|x|, per-head att dot (VectorE reduce), exp on ScalarE, and
     gpsimd.dma_scatter_add of [a*xl_s | a] payloads = fused segment-sum.
  C. h = relu(mean_h(num/den) + bias).
  D. TopK per graph without sorting: rank i by count_j[s_j > s_i or
     (s_j == s_i and j < i)] < k  (exactly jax.lax.top_k tie semantics),
     computed as O(N^2) vector compares against a PE-replicated score row.
  E. Masked global max/mean pool via PE transposes + free-dim reductions.

Self-contained: hardcodes all shapes; no repo-local imports.
"""
import numpy as np
import jax

import concourse.bacc as bacc
import concourse.mybir as mybir
import concourse.tile as tile
import concourse.bass_utils as bass_utils

F32 = mybir.dt.float32
I16 = mybir.dt.int16
I32 = mybir.dt.int32
ALU = mybir.AluOpType
ACTF = mybir.ActivationFunctionType
AX = mybir.AxisListType

# ---- problem config (full size) ----
FULL_CFG = dict(CORES=8, NG=4, N=1024, DEG=8, F=64, EDIM=16, H=128, HD=4,
                K1=820, K2=656, ET=512, NEG=0.2)
NR = 16
B = 32
ROW_PAD = 64  # payload row = HC + ROW_PAD (256B DMA elem granularity)


def _derived(cfg):
    d = dict(cfg)
    d["HC"] = cfg["HD"] * cfg["H"]
    d["NL"] = cfg["NG"] * cfg["N"]
    d["EL"] = d["NL"] * cfg["DEG"]
    d["NT"] = d["NL"] // 128
    d["GT"] = cfg["N"] // 128
    d["C"] = cfg["ET"] // 128
    d["ETI"] = d["EL"] // cfg["ET"]
    d["ROW"] = d["HC"] + ROW_PAD
    assert cfg["N"] % 128 == 0 and cfg["ET"] % 128 == 0 and d["EL"] % cfg["ET"] == 0
    assert d["NL"] % 256 == 0 and d["EL"] % 1024 == 0
    return d


def _build_program(cfg, debug=False, do_compile=True):
    d = _derived(cfg)
    NG, N, F, EDIM, H, HD = (cfg["NG"], cfg["N"], cfg["F"], cfg["EDIM"],
                             cfg["H"], cfg["HD"])
    HC, NL, EL, NT, GT, C, ETI, ROW = (d["HC"], d["NL"], d["EL"], d["NT"],
                                       d["GT"], d["C"], d["ETI"], d["ROW"])
    ET = cfg["ET"]
    K = [float(cfg["K1"]), float(cfg["K2"])]
    LR_A = 0.5 * (1.0 + cfg["NEG"])
    LR_B = 0.5 * (1.0 - cfg["NEG"])

    nc = bacc.Bacc("TRN2", target_bir_lowering=False, debug=False)

    # ---- external inputs (per core) ----
    xT_q = nc.dram_tensor("xT_q", [F, NL], I16, kind="ExternalInput")
    eaT_q = nc.dram_tensor("eaT_q", [EDIM, EL], I16, kind="ExternalInput")
    srcw = nc.dram_tensor("srcw", [16, EL // 16], I16, kind="ExternalInput")
    dstw = nc.dram_tensor("dstw", [16, EL // 16], I16, kind="ExternalInput")
    w1l_q = nc.dram_tensor("w1l_q", [F, HC], I16, kind="ExternalInput")
    w1r_q = nc.dram_tensor("w1r_q", [F, HC], I16, kind="ExternalInput")
    w2l_q = nc.dram_tensor("w2l_q", [H, HC], I16, kind="ExternalInput")
    w2r_q = nc.dram_tensor("w2r_q", [H, HC], I16, kind="ExternalInput")
    w1e_q = nc.dram_tensor("w1e_q", [EDIM, HC], I16, kind="ExternalInput")
    w2e_q = nc.dram_tensor("w2e_q", [EDIM, HC], I16, kind="ExternalInput")
    smalls = nc.dram_tensor("smalls", [16, HC], F32, kind="ExternalInput")
    scales = nc.dram_tensor("scales", [16, 16], F32, kind="ExternalInput")

    outp = nc.dram_tensor("outp", [NG, 2 * H], F32, kind="ExternalOutput")
    if debug:
        dbg_h1 = nc.dram_tensor("dbg_h1", [NL, H], F32, kind="ExternalOutput")
        dbg_h2 = nc.dram_tensor("dbg_h2", [NL, H], F32, kind="ExternalOutput")
        dbg_cols = nc.dram_tensor("dbg_cols", [128, 8 * NT], F32,
                                  kind="ExternalOutput")
        dbg_xl = nc.dram_tensor("dbg_xl", [NL, ROW], F32, kind="ExternalOutput")
        dbg_hb = nc.dram_tensor("dbg_hb", [NL, ROW], F32, kind="ExternalOutput")

    xl_hbm = nc.dram_tensor("xl_hbm", [NL, ROW], F32, kind="Internal")
    xr_hbm = nc.dram_tensor("xr_hbm", [NL, ROW], F32, kind="Internal")
    h_hbm = nc.dram_tensor("h_hbm", [NL, ROW], F32, kind="Internal")

    iota_eng = nc.gpsimd if hasattr(nc.gpsimd, "iota") else nc.vector

    with tile.TileContext(nc) as tc:
        with tc.tile_pool(name="persist", bufs=1) as P:
            # ---------- phase 0 ----------
            srcw_sb = P.tile([128, EL // 16], I16, tag="srcw")
            dstw_sb = P.tile([128, EL // 16], I16, tag="dstw")
            for k in range(8):
                nc.sync.dma_start(out=srcw_sb[16 * k:16 * k + 16, :], in_=srcw[:, :])
                nc.sync.dma_start(out=dstw_sb[16 * k:16 * k + 16, :], in_=dstw[:, :])
            smalls_sb = P.tile([16, HC], F32, tag="smalls")
            nc.sync.dma_start(out=smalls_sb[:], in_=smalls[:, :])
            scal_rep = P.tile([128, 16], F32, tag="scal_rep")
            for k in range(8):
                nc.sync.dma_start(out=scal_rep[16 * k:16 * k + 16, :],
                                  in_=scales[:, :])

            with tc.tile_pool(name="stage", bufs=2) as STP, \
                 tc.tile_pool(name="p0ps", bufs=2, space="PSUM") as P0P:

                def dequant(dram, shape, scale_idx, tag):
                    st_t = STP.tile(shape, I16, tag="stg")
                    nc.sync.dma_start(out=st_t[:], in_=dram[:, :])
                    out_t = P.tile(shape, F32, tag=tag)
                    nc.vector.tensor_scalar(
                        out=out_t[:], in0=st_t[:],
                        scalar1=scal_rep[0:shape[0], scale_idx:scale_idx + 1],
                        scalar2=None, op0=ALU.mult)
                    return out_t

                # xT shares its slot with the later hgT (dead after phase A-1)
                xT2_full = P.tile([128, NL], F32, tag="xT2_hgT")
                xT = xT2_full[0:F, 0:NL]
                st_x = STP.tile([F, NL], I16, tag="stg")
                nc.sync.dma_start(out=st_x[:], in_=xT_q[:, :])
                nc.vector.tensor_scalar(out=xT, in0=st_x[:],
                                        scalar1=scal_rep[0:F, 0:1],
                                        scalar2=None, op0=ALU.mult)
                w1l = dequant(w1l_q, [F, HC], 2, "w1l")
                w1r = dequant(w1r_q, [F, HC], 3, "w1r")
                w2l = dequant(w2l_q, [H, HC], 4, "w2l")
                w2r = dequant(w2r_q, [H, HC], 5, "w2r")
                w1e = dequant(w1e_q, [EDIM, HC], 6, "w1e")
                w2e = dequant(w2e_q, [EDIM, HC], 7, "w2e")

                ones_row = P.tile([1, 128], F32, tag="ones_row")
                nc.vector.memset(ones_row[:], 1.0)
                io_row = STP.tile([128, 128], I32, tag="io_row")
                iota_eng.iota(io_row[:], pattern=[[1, 128]], base=0,
                              channel_multiplier=0)
                io_col = STP.tile([128, 1], I32, tag="io_col")
                iota_eng.iota(io_col[:], pattern=[[0, 1]], base=0,
                              channel_multiplier=1)
                io_rowf = STP.tile([128, 128], F32, tag="io_rowf")
                nc.vector.tensor_copy(out=io_rowf[:], in_=io_row[:])
                io_colf = STP.tile([128, 1], F32, tag="io_colf")
                nc.vector.tensor_copy(out=io_colf[:], in_=io_col[:])
                ident = P.tile([128, 128], F32, tag="ident")
                nc.vector.tensor_scalar(out=ident[:], in0=io_rowf[:],
                                        scalar1=io_colf[:, 0:1], scalar2=None,
                                        op0=ALU.is_equal)
                jrow_i = STP.tile([128, N], I32, tag="jrow_i")
                iota_eng.iota(jrow_i[:], pattern=[[1, N]], base=0,
                              channel_multiplier=0)
                jrow = P.tile([128, N], F32, tag="jrow")
                nc.vector.tensor_copy(out=jrow[:], in_=jrow_i[:])
                icol_i = STP.tile([128, GT], I32, tag="icol_i")
                iota_eng.iota(icol_i[:], pattern=[[128, GT]], base=0,
                              channel_multiplier=1)
                icol = P.tile([128, GT], F32, tag="icol")
                nc.vector.tensor_copy(out=icol[:], in_=icol_i[:])

                def replicate(row, off, width, tag):
                    out_t = P.tile([128, width], F32, tag=tag)
                    row_t = STP.tile([1, HC], F32, tag="rep_row")
                    nc.sync.dma_start(out=row_t[0:1, 0:width],
                                      in_=smalls[row:row + 1, off:off + width])
                    for j in range(0, width, 512):
                        w = min(512, width - j)
                        ps = P0P.tile([128, 512], F32, tag="rep_ps")
                        nc.tensor.matmul(
                            ps[:, 0:w], lhsT=ones_row[:],
                            rhs=row_t[0:1, j:j + w],
                            start=True, stop=True)
                        nc.vector.tensor_copy(out=out_t[:, j:j + w], in_=ps[:, 0:w])
                    return out_t

                b1l_r = replicate(0, 0, HC, "b1l_r")
                b1r_r = replicate(1, 0, HC, "b1r_r")
                b2l_r = replicate(2, 0, HC, "b2l_r")
                b2r_r = replicate(3, 0, HC, "b2r_r")
                att1_r = replicate(4, 0, HC, "att1_r")
                att2_r = replicate(5, 0, HC, "att2_r")
                bias1_r = replicate(6, 0, H, "bias1_r")
                bias2_r = replicate(7, 0, H, "bias2_r")
                p1_r = replicate(8, 0, H, "p1_r")
                p2_r = replicate(9, 0, H, "p2_r")

            zt = P.tile([128, ROW], F32, tag="zt")
            nc.vector.memset(zt[:], 0.0)

            # h1 and h2 share one slot (h1 dead after the hgT transpose)
            keep_col = [P.tile([128, NT], F32, tag="keep1", name="keep1"),
                        P.tile([128, NT], F32, tag="keep2", name="keep2")]
            mb_col = P.tile([128, NT], F32, tag="mb_col")
            s_col_t = [P.tile([128, NT], F32, tag="s_col1", name="s_col1"),
                       P.tile([128, NT], F32, tag="s_col2", name="s_col2")]
            cnt_col_t = [P.tile([128, NT], F32, tag="cnt1", name="cnt1"),
                         P.tile([128, NT], F32, tag="cnt2", name="cnt2")]
            gate_col_t = [P.tile([128, NT], F32, tag="gate1", name="gate1"),
                          P.tile([128, NT], F32, tag="gate2", name="gate2")]
            dbg_cols_sb = P.tile([128, 8 * NT], F32, tag="dbgc", name="dbgc") if debug else None

            h_prev = None
            hgT = None
            for layer in range(2):
                wl, wr, we = (w1l, w1r, w1e) if layer == 0 else (w2l, w2r, w2e)
                bl_r, br_r = (b1l_r, b1r_r) if layer == 0 else (b2l_r, b2r_r)
                att_r = att1_r if layer == 0 else att2_r
                bias_r = bias1_r if layer == 0 else bias2_r
                p_r = p1_r if layer == 0 else p2_r
                h_t = P.tile([128, NT, H], F32, tag="h_sb")
                s_col = s_col_t[layer]
                cnt_col = cnt_col_t[layer]
                gate_col = gate_col_t[layer]

                # ---------- phase A ----------
                with tc.tile_pool(name=f"pa{layer}", bufs=3) as PA, \
                     tc.tile_pool(name=f"pap{layer}", bufs=4, space="PSUM") as PAP:
                    for nt in range(NT):
                        if layer == 0:
                            lhsT = xT[0:F, nt * 128:(nt + 1) * 128]
                        else:
                            lhsT = hgT[:, nt * 128:(nt + 1) * 128]
                        for (tbl, w_t, b_t) in ((xl_hbm, wl, bl_r),
                                                (xr_hbm, wr, br_r)):
                            ps = PAP.tile([128, HC], F32, tag="ps")
                            nc.tensor.matmul(ps[:], lhsT=lhsT, rhs=w_t[:],
                                             start=True, stop=True)
                            xt = PA.tile([128, ROW], F32, tag="xt")
                            nc.vector.tensor_tensor(out=xt[:, 0:HC], in0=ps[:],
                                                    in1=b_t[:], op=ALU.add)
                            if layer == 0:
                                nc.vector.memset(xt[:, HC:ROW], 0.0)
                            else:
                                nc.vector.tensor_copy(out=xt[:, HC:HC + 1],
                                                      in_=mb_col[:, nt:nt + 1])
                                nc.vector.memset(xt[:, HC + 1:ROW], 0.0)
                            nc.sync.dma_start(
                                out=tbl[nt * 128:(nt + 1) * 128, :], in_=xt[:])
                            if debug and layer == 0 and tbl is xl_hbm:
                                nc.sync.dma_start(
                                    out=dbg_xl[nt * 128:(nt + 1) * 128, :],
                                    in_=xt[:])
                        nc.sync.dma_start(out=h_hbm[nt * 128:(nt + 1) * 128, :],
                                          in_=zt[:])

                # ---------- phase B: edge loop ----------
                with tc.tile_pool(name=f"pb{layer}", bufs=2) as PB, \
                     tc.tile_pool(name=f"pb1{layer}", bufs=1) as PB1, \
                     tc.tile_pool(name=f"pbp{layer}", bufs=2, space="PSUM") as PBP:
                    for et in range(ETI):
                        i0, i1 = et * (ET // 16), (et + 1) * (ET // 16)
                        xlg = PB.tile([128, C, ROW], F32, tag="xlg")
                        nc.gpsimd.dma_gather(
                            out_ap=xlg[:], in_ap=xl_hbm[:, :],
                            idxs_ap=srcw_sb[:, i0:i1], num_idxs=ET,
                            num_idxs_reg=ET, elem_size=ROW)
                        xrg = PB.tile([128, C, ROW], F32, tag="xrg")
                        nc.gpsimd.dma_gather(
                            out_ap=xrg[:], in_ap=xr_hbm[:, :],
                            idxs_ap=dstw_sb[:, i0:i1], num_idxs=ET,
                            num_idxs_reg=ET, elem_size=ROW)
                        ea_st = PB.tile([EDIM, ET], I16, tag="ea_st")
                        nc.sync.dma_start(out=ea_st[:],
                                          in_=eaT_q[:, et * ET:(et + 1) * ET])
                        ea_f = PB.tile([EDIM, ET], F32, tag="ea_f")
                        nc.vector.tensor_scalar(out=ea_f[:], in0=ea_st[:],
                                                scalar1=scal_rep[0:EDIM, 1:2],
                                                scalar2=None, op0=ALU.mult)
                        efp = PBP.tile([128, C, HC], F32, tag="efp")
                        for k in range(C):
                            nc.tensor.matmul(
                                efp[:, k, :],
                                lhsT=ea_f[:, k * 128:(k + 1) * 128],
                                rhs=we[:], start=True, stop=True)
                        m_t = PB1.tile([128, C, HC], F32, tag="m_t")
                        nc.vector.tensor_tensor(out=m_t[:], in0=xlg[:, :, 0:HC],
                                                in1=xrg[:, :, 0:HC], op=ALU.add)
                        nc.vector.tensor_tensor(out=m_t[:], in0=m_t[:],
                                                in1=efp[:], op=ALU.add)
                        am = PB1.tile([128, C, HC], F32, tag="am")
                        nc.scalar.activation(out=am[:], in_=m_t[:],
                                             func=ACTF.Abs, scale=LR_B)
                        nc.vector.scalar_tensor_tensor(
                            out=m_t[:], in0=m_t[:], scalar=LR_A, in1=am[:],
                            op0=ALU.mult, op1=ALU.add)
                        nc.vector.tensor_tensor(out=m_t[:], in0=m_t[:],
                                                in1=att_r[:].unsqueeze(1)
                                                .broadcast_to((128, C, HC)),
                                                op=ALU.mult)
                        lg = PB.tile([128, C, HD], F32, tag="lg")
                        nc.vector.tensor_reduce(
                            out=lg[:],
                            in_=m_t[:].rearrange("p c (h d) -> p c h d", h=HD),
                            axis=AX.X, op=ALU.add)
                        mbt = PB.tile([128, C, 1], F32, tag="mbt")
                        nc.vector.tensor_tensor(out=mbt[:],
                                                in0=xlg[:, :, HC:HC + 1],
                                                in1=xrg[:, :, HC:HC + 1],
                                                op=ALU.add)
                        for k in range(C):
                            nc.vector.tensor_scalar(
                                out=lg[:, k, :], in0=lg[:, k, :],
                                scalar1=mbt[:, k, 0:1], scalar2=None,
                                op0=ALU.add)
                        a4 = PB.tile([128, C, HD], F32, tag="a4")
                        nc.scalar.activation(out=a4[:], in_=lg[:], func=ACTF.Exp)
                        pay = PB.tile([128, C, ROW], F32, tag="pay")
                        nc.vector.tensor_tensor(
                            out=pay[:, :, 0:HC].rearrange(
                                "p c (h d) -> p c h d", h=HD),
                            in0=xlg[:, :, 0:HC].rearrange(
                                "p c (h d) -> p c h d", h=HD),
                            in1=a4[:].unsqueeze(3).broadcast_to((128, C, HD, H)),
                            op=ALU.mult)
                        nc.vector.tensor_copy(out=pay[:, :, HC:HC + HD],
                                              in_=a4[:])
                        nc.vector.memset(pay[:, :, HC + HD:ROW], 0.0)
                        nc.gpsimd.dma_scatter_add(
                            h_hbm[:, :], pay[:], dstw_sb[:, i0:i1],
                            ET, ET, ROW)

                # ---------- phase C ----------
                with tc.tile_pool(name=f"pc{layer}", bufs=3) as PC:
                    for nt in range(NT):
                        ht = PC.tile([128, ROW], F32, tag="ht")
                        nc.sync.dma_start(out=ht[:],
                                          in_=h_hbm[nt * 128:(nt + 1) * 128, :])
                        if debug and layer == 0:
                            nc.sync.dma_start(
                                out=dbg_hb[nt * 128:(nt + 1) * 128, :], in_=ht[:])
                        d4 = PC.tile([128, HD], F32, tag="d4")
                        nc.vector.tensor_scalar(
                            out=d4[:], in0=ht[:, HC:HC + HD],
                            scalar1=float(HD), scalar2=float(HD) * 1e-16,
                            op0=ALU.mult, op1=ALU.add)
                        dr = PC.tile([128, HD], F32, tag="dr")
                        nc.vector.reciprocal(out=dr[:], in_=d4[:])
                        hn = h_t[:, nt, :]
                        tmp = PC.tile([128, H], F32, tag="ctmp")
                        for hh in range(HD):
                            tgt = hn if hh == 0 else tmp[:]
                            nc.vector.tensor_scalar(
                                out=tgt, in0=ht[:, hh * H:(hh + 1) * H],
                                scalar1=dr[:, hh:hh + 1], scalar2=None,
                                op0=ALU.mult)
                            if hh > 0:
                                nc.vector.tensor_tensor(out=hn, in0=hn,
                                                        in1=tmp[:], op=ALU.add)
                        nc.vector.tensor_tensor(out=hn, in0=hn, in1=bias_r[:],
                                                op=ALU.add)
                        nc.vector.tensor_scalar(out=hn, in0=hn, scalar1=0.0,
                                                scalar2=None, op0=ALU.max)
                        if debug:
                            dbg_h = dbg_h1 if layer == 0 else dbg_h2
                            nc.sync.dma_start(
                                out=dbg_h[nt * 128:(nt + 1) * 128, :], in_=hn)

                # ---------- phase D: topk ----------
                with tc.tile_pool(name=f"pd{layer}", bufs=2) as PD, \
                     tc.tile_pool(name=f"pd1{layer}", bufs=1) as PD1, \
                     tc.tile_pool(name=f"pdp{layer}", bufs=2, space="PSUM") as PDP:
                    for nt in range(NT):
                        tmp = PD.tile([128, H], F32, tag="sd_tmp")
                        nc.vector.tensor_tensor(out=tmp[:], in0=h_t[:, nt, :],
                                                in1=p_r[:], op=ALU.mult)
                        nc.vector.tensor_reduce(out=s_col[:, nt:nt + 1],
                                                in_=tmp[:], axis=AX.X,
                                                op=ALU.add)
                    nc.scalar.activation(out=gate_col[:], in_=s_col[:],
                                         func=ACTF.Tanh)
                    if layer == 0:
                        sm_col = s_col
                    else:
                        sm_col = PD1.tile([128, NT], F32, tag="sm_col")
                        nc.vector.tensor_scalar(out=sm_col[:],
                                                in0=keep_col[0][:],
                                                scalar1=1.0, scalar2=1e30,
                                                op0=ALU.subtract, op1=ALU.mult)
                        nc.vector.tensor_tensor(out=sm_col[:], in0=sm_col[:],
                                                in1=s_col[:], op=ALU.add)
                    psT = PDP.tile([128, 128], F32, tag="psT")
                    nc.tensor.matmul(psT[0:NT, :], lhsT=sm_col[:], rhs=ident[:],
                                     start=True, stop=True)
                    sT = PD1.tile([NT, 128], F32, tag="sT")
                    nc.vector.tensor_copy(out=sT[:], in_=psT[0:NT, :])
                    srow = PD1.tile([1, NL], F32, tag="srow")
                    for t in range(NT):
                        nc.sync.dma_start(out=srow[0:1, t * 128:(t + 1) * 128],
                                          in_=sT[t:t + 1, :])
                    srep = PD1.tile([128, NL], F32, tag="srep")
                    for j in range(0, NL, 512):
                        ps = PDP.tile([128, 512], F32, tag="srep_ps")
                        nc.tensor.matmul(ps[:], lhsT=ones_row[:],
                                         rhs=srow[:, j:j + 512],
                                         start=True, stop=True)
                        nc.vector.tensor_copy(out=srep[:, j:j + 512], in_=ps[:])
                    for g in range(NG):
                        for tl in range(GT):
                            nt = g * GT + tl
                            gsl = srep[:, g * N:(g + 1) * N]
                            cgt = PD.tile([128, N], F32, tag="cgt")
                            nc.vector.tensor_scalar(
                                out=cgt[:], in0=gsl, scalar1=sm_col[:, nt:nt + 1],
                                scalar2=None, op0=ALU.is_gt)
                            ceq = PD.tile([128, N], F32, tag="ceq")
                            nc.vector.tensor_scalar(
                                out=ceq[:], in0=gsl, scalar1=sm_col[:, nt:nt + 1],
                                scalar2=None, op0=ALU.is_equal)
                            clt = PD.tile([128, N], F32, tag="clt")
                            nc.vector.tensor_scalar(
                                out=clt[:], in0=jrow[:],
                                scalar1=icol[:, tl:tl + 1],
                                scalar2=None, op0=ALU.is_lt)
                            nc.vector.tensor_tensor(out=ceq[:], in0=ceq[:],
                                                    in1=clt[:], op=ALU.mult)
                            nc.vector.tensor_tensor(out=cgt[:], in0=cgt[:],
                                                    in1=ceq[:], op=ALU.add)
                            nc.vector.tensor_reduce(
                                out=cnt_col[:, nt:nt + 1], in_=cgt[:],
                                axis=AX.X, op=ALU.add)
                    nc.vector.tensor_scalar(out=keep_col[layer][:],
                                            in0=cnt_col[:], scalar1=K[layer],
                                            scalar2=None, op0=ALU.is_lt)
                    for nt in range(NT):
                        nc.vector.tensor_scalar(
                            out=h_t[:, nt, :], in0=h_t[:, nt, :],
                            scalar1=gate_col[:, nt:nt + 1], scalar2=None,
                            op0=ALU.mult)
                    if layer == 0:
                        nc.vector.tensor_scalar(out=mb_col[:],
                                                in0=keep_col[0][:],
                                                scalar1=1.0, scalar2=1e9,
                                                op0=ALU.subtract, op1=ALU.mult)
                        hgT = P.tile([128, NL], F32, tag="xT2_hgT")
                        for nt in range(NT):
                            pt = PDP.tile([128, 128], F32, tag="tr_ps")
                            nc.tensor.matmul(pt[:], lhsT=h_t[:, nt, :],
                                             rhs=ident[:], start=True, stop=True)
                            nc.vector.tensor_copy(
                                out=hgT[:, nt * 128:(nt + 1) * 128], in_=pt[:])
                h_prev = h_t

            if debug:
                for i, t in enumerate([s_col_t[0], cnt_col_t[0], keep_col[0],
                                       gate_col_t[0], s_col_t[1], cnt_col_t[1],
                                       keep_col[1], gate_col_t[1]]):
                    nc.vector.tensor_copy(
                        out=dbg_cols_sb[:, i * NT:(i + 1) * NT], in_=t[:])
                nc.sync.dma_start(out=dbg_cols[:, :], in_=dbg_cols_sb[:])

            # ---------- phase E: pooled readout ----------
            h2_t = h_prev
            with tc.tile_pool(name="pe", bufs=2) as PE_, \
                 tc.tile_pool(name="pe1", bufs=1) as PE1, \
                 tc.tile_pool(name="pep", bufs=2, space="PSUM") as PEP:
                negm = PE1.tile([128, NT], F32, tag="negm")
                nc.vector.tensor_scalar(out=negm[:], in0=keep_col[1][:],
                                        scalar1=1.0, scalar2=1e30,
                                        op0=ALU.subtract, op1=ALU.mult)
                gmxT = PE1.tile([128, NG], F32, tag="gmxT")
                gavT = PE1.tile([128, NG], F32, tag="gavT")
                for g in range(NG):
                    accx = PE1.tile([128, 128], F32, tag="accx")
                    accv = PE1.tile([128, 128], F32, tag="accv")
                    for tl in range(GT):
                        nt = g * GT + tl
                        ms = PE_.tile([128, 128], F32, tag="ms")
                        nc.vector.tensor_scalar(
                            out=ms[:], in0=h2_t[:, nt, :],
                            scalar1=negm[:, nt:nt + 1], scalar2=None,
                            op0=ALU.add)
                        ptx = PEP.tile([128, 128], F32, tag="ptx")
                        nc.tensor.matmul(ptx[:], lhsT=ms[:], rhs=ident[:],
                                         start=True, stop=True)
                        mv = PE_.tile([128, 128], F32, tag="mv")
                        nc.vector.tensor_scalar(
                            out=mv[:], in0=h2_t[:, nt, :],
                            scalar1=keep_col[1][:, nt:nt + 1], scalar2=None,
                            op0=ALU.mult)
                        ptv = PEP.tile([128, 128], F32, tag="ptv")
                        nc.tensor.matmul(ptv[:], lhsT=mv[:], rhs=ident[:],
                                         start=True, stop=True)
                        if tl == 0:
                            nc.vector.tensor_copy(out=accx[:], in_=ptx[:])
                            nc.vector.tensor_copy(out=accv[:], in_=ptv[:])
                        else:
                            nc.vector.tensor_tensor(out=accx[:], in0=accx[:],
                                                    in1=ptx[:], op=ALU.max)
                            nc.vector.tensor_tensor(out=accv[:], in0=accv[:],
                                                    in1=ptv[:], op=ALU.add)
                    nc.vector.tensor_reduce(out=gmxT[:, g:g + 1], in_=accx[:],
                                            axis=AX.X, op=ALU.max)
                    nc.vector.tensor_reduce(out=gavT[:, g:g + 1], in_=accv[:],
                                            axis=AX.X, op=ALU.add)
                out_sb = PE1.tile([NG, 2 * H], F32, tag="out_sb")
                pso = PEP.tile([128, 128], F32, tag="ptx")
                nc.tensor.matmul(pso[0:NG, :], lhsT=gmxT[:], rhs=ident[:],
                                 start=True, stop=True)
                nc.vector.tensor_copy(out=out_sb[:, 0:H], in_=pso[0:NG, :])
                pso2 = PEP.tile([128, 128], F32, tag="ptv")
                nc.tensor.matmul(pso2[0:NG, :], lhsT=gavT[:], rhs=ident[:],
                                 start=True, stop=True)
                nc.vector.tensor_scalar(out=out_sb[:, H:2 * H],
                                        in0=pso2[0:NG, :],
                                        scalar1=1.0 / cfg["K2"], scalar2=None,
                                        op0=ALU.mult)
                nc.sync.dma_start(out=outp[:, :], in_=out_sb[:])

    if do_compile:
        nc.compile()
    return nc


# ---------------- host side ----------------

def _pow2_scale(arr):
    """Power-of-two scale s with |arr|/s comfortably inside int16."""
    m = float(np.max(np.abs(arr))) if arr.size else 0.0
    if m == 0.0 or not np.isfinite(m):
        m = 1.0
    e = int(np.ceil(np.log2(m)))
    return float(2.0 ** (e - 14))


def _quant(arr, s):
    return np.clip(np.rint(arr * (1.0 / s)), -32767, 32767).astype(np.int16)


def _pack_xT(xq, NL, F):
    return np.ascontiguousarray(xq.T)


def _pack_eaT(eq, EL, EDIM):
    return np.ascontiguousarray(eq.T)


def _wrap_idx(idx):
    return np.ascontiguousarray(idx.reshape(-1, 16).T)


_CACHE = {}


def _get_runner(cfg, scale_key, use_spmd_first):
    """Build+compile the program for this pow2-scale bucket and return a
    cached dispatcher. The first dispatch goes through
    bass_utils.run_bass_kernel_spmd (which also JITs the NEFF); subsequent
    dispatches reuse a cached jax.jit of the identical _bass_exec_p call
    graph to skip per-call retracing."""
    key = scale_key
    if key in _CACHE:
        return _CACHE[key]
    nc = _build_program(cfg, debug=False)

    from concourse import bass2jax
    from jax.sharding import Mesh, PartitionSpec
    from jax.experimental.shard_map import shard_map

    n_cores = cfg["CORES"]
    state = {"jit": None, "meta": None}

    def runner(in_maps):
        if state["jit"] is None:
            # first call: run through the stock bass_utils path (compiles NEFF)
            res = bass_utils.run_bass_kernel_spmd(
                nc, in_maps, core_ids=list(range(n_cores)), trace=False)
            out0 = [res.results[c]["outp"] for c in range(n_cores)]

            # build the cached jit for steady-state calls
            bass2jax.install_neuronx_cc_hook()
            pn = (nc.partition_id_tensor.name
                  if nc.partition_id_tensor is not None else None)
            in_names, out_names, out_avals, zshapes = [], [], [], []
            for alloc in nc.m.functions[0].allocations:
                if not isinstance(alloc, mybir.MemoryLocationSet):
                    continue
                name = alloc.memorylocations[0].name
                if alloc.kind == "ExternalInput":
                    if name != pn:
                        in_names.append(name)
                elif alloc.kind == "ExternalOutput":
                    out_names.append(name)
                    shape = tuple(alloc.tensor_shape)
                    dtype = mybir.dt.np(alloc.dtype)
                    out_avals.append(jax.core.ShapedArray(shape, dtype))
                    zshapes.append((shape, dtype))
            all_names = tuple(in_names) + tuple(out_names) + \
                ((pn,) if pn else ())
            n_params = len(in_names)
            extra = {}
            if nc.dbg_addr is not None:
                extra[nc.dbg_addr.name] = np.zeros((1, 2), np.uint32)
                # dbg_addr is an ExternalInput already listed in in_names

            def _body(*args):
                operands = list(args)
                if pn is not None:
                    operands.append(bass2jax.partition_id_tensor())
                return tuple(bass2jax._bass_exec_p.bind(
                    *operands, out_avals=tuple(out_avals),
                    in_names=all_names, out_names=tuple(out_names),
                    lowering_input_output_aliases=(),
                    sim_require_finite=True, sim_require_nnan=True, nc=nc))

            mesh = Mesh(np.asarray(jax.devices()[:n_cores]), ("core",))
            n_outs = len(out_names)
            sharded = jax.jit(
                shard_map(_body, mesh=mesh,
                          in_specs=(PartitionSpec("core"),) * (n_params + n_outs),
                          out_specs=(PartitionSpec("core"),) * n_outs,
                          check_rep=False),
                donate_argnums=tuple(range(n_params, n_params + n_outs)),
                keep_unused=True)
            state["jit"] = sharded
            state["meta"] = (in_names, out_names, zshapes, extra)
            return out0

        sharded = state["jit"]
        in_names, out_names, zshapes, extra = state["meta"]
        concat_in = [
            np.concatenate([np.asarray(m.get(nm, extra.get(nm)))
                            for m in in_maps], axis=0)
            for nm in in_names]
        zeros = [np.zeros((n_cores * s[0],) + tuple(s[1:]), dt)
                 for (s, dt) in zshapes]
        outs = sharded(*concat_in, *zeros)
        oi = out_names.index("outp")
        arr = np.asarray(outs[oi])
        per = arr.reshape(n_cores, -1, arr.shape[-1])
        return [per[c] for c in range(n_cores)]

    _CACHE[key] = (nc, runner)
    return _CACHE[key]


def kernel(x, edge_attr, action, W1l, b1l, W1r, b1r, W1e, att1, bias1,
           W2l, b2l, W2r, b2r, W2e, att2, bias2, p1, p2,
           Wf1, bf1, Wf2, bf2, Wf3, bf3, edge_index):
    cfg = FULL_CFG
    d = _derived(cfg)
    CORES, NG, N = cfg["CORES"], cfg["NG"], cfg["N"]
    F, EDIM, H, HD = cfg["F"], cfg["EDIM"], cfg["H"], cfg["HD"]
    HC, NL, EL = d["HC"], d["NL"], d["EL"]

    f32 = np.float32
    x = np.asarray(x, f32)
    edge_attr = np.asarray(edge_attr, f32)
    action = np.asarray(action, f32)
    edge_index = np.asarray(edge_index, np.int64)

    Ws = {k: np.asarray(v, f32) for k, v in dict(
        W1l=W1l, W1r=W1r, W2l=W2l, W2r=W2r, W1e=W1e, W2e=W2e).items()}
    sm = {k: np.asarray(v, f32) for k, v in dict(
        b1l=b1l, b1r=b1r, b2l=b2l, b2r=b2r, att1=att1, att2=att2,
        bias1=bias1, bias2=bias2, p1=p1, p2=p2).items()}

    # quantization scales (pow2 -> compile cache key stable across calls)
    sx = _pow2_scale(x)
    se = _pow2_scale(edge_attr)
    sw = [_pow2_scale(Ws[k]) for k in ("W1l", "W1r", "W2l", "W2r", "W1e", "W2e")]
    scale_vec = np.zeros(16, f32)
    scale_vec[0], scale_vec[1] = sx, se
    scale_vec[2:8] = sw
    scales_arr = np.ascontiguousarray(np.tile(scale_vec, (16, 1)))

    wq = {}
    for i, k in enumerate(("W1l", "W1r", "W2l", "W2r", "W1e", "W2e")):
        wq[k] = _quant(Ws[k], sw[i])

    smalls_arr = np.zeros((16, HC), f32)
    smalls_arr[0, :HC] = np.broadcast_to(sm["b1l"], (HC,))
    smalls_arr[1, :HC] = np.broadcast_to(sm["b1r"], (HC,))
    smalls_arr[2, :HC] = np.broadcast_to(sm["b2l"], (HC,))
    smalls_arr[3, :HC] = np.broadcast_to(sm["b2r"], (HC,))
    smalls_arr[4, :HC] = sm["att1"].reshape(-1)
    smalls_arr[5, :HC] = sm["att2"].reshape(-1)
    smalls_arr[6, :H] = np.broadcast_to(sm["bias1"], (H,))
    smalls_arr[7, :H] = np.broadcast_to(sm["bias2"], (H,))
    smalls_arr[8, :H] = sm["p1"] / (np.linalg.norm(sm["p1"]) + 1e-16)
    smalls_arr[9, :H] = sm["p2"] / (np.linalg.norm(sm["p2"]) + 1e-16)

    xq_all = _quant(x, sx)
    eq_all = _quant(edge_attr, se)
    src_all = edge_index[0]
    dst_all = edge_index[1]

    in_maps = []
    for c in range(CORES):
        nsl = slice(c * NL, (c + 1) * NL)
        esl = slice(c * EL, (c + 1) * EL)
        src_l = (src_all[esl] - c * NL).astype(np.int16)
        dst_l = (dst_all[esl] - c * NL).astype(np.int16)
        in_maps.append({
            "xT_q": _pack_xT(xq_all[nsl], NL, F),
            "eaT_q": _pack_eaT(eq_all[esl], EL, EDIM),
            "srcw": _wrap_idx(src_l),
            "dstw": _wrap_idx(dst_l),
            "w1l_q": wq["W1l"], "w1r_q": wq["W1r"],
            "w2l_q": wq["W2l"], "w2r_q": wq["W2r"],
            "w1e_q": wq["W1e"], "w2e_q": wq["W2e"],
            "smalls": smalls_arr, "scales": scales_arr,
        })

    scale_key = tuple(np.log2(scale_vec[:8]).astype(int))
    _, runner = _get_runner(cfg, scale_key, True)
    outs = runner(in_maps)  # list of [NG, 2H] per core

    pooled = np.concatenate(outs, axis=0).astype(f32)  # [B, 2H]
    z = np.concatenate([pooled, action], axis=1)
    z = np.maximum(z @ np.asarray(Wf1, f32) + np.asarray(bf1, f32), 0)
    z = np.maximum(z @ np.asarray(Wf2, f32) + np.asarray(bf2, f32), 0)
    return (z @ np.asarray(Wf3, f32) + np.asarray(bf3, f32)).astype(f32)
